# revision 1
# baseline (speedup 1.0000x reference)
"""AFNO2D layer distributed across 8 Trainium2 NeuronCores.

Sharding: the block-diagonal channel MLP has NUM_BLOCKS=8 independent
96-channel blocks, and the 2D FFT is independent per channel — so each
core takes one block (96 channels) end-to-end with zero collectives.

The rfft2/irfft2 are expressed as real matmuls against precomputed DFT
matrices (cos/sin), so the whole per-shard computation lowers to dense
matmuls + elementwise ops on the NeuronCore.
"""

import numpy as np

H = 256
W = 256
HIDDEN = 768
NB = 8          # num blocks == num cores
BS = 96         # block size (channels per core)
WC = W // 2 + 1  # 129 rfft bins
LAMBDA = 0.01
N_CORES = 8


def _dft_mats():
    n = np.arange(H)
    k = np.arange(H)
    theta = 2.0 * np.pi * np.outer(n, k) / H
    scale = 1.0 / np.sqrt(H)
    # forward kernel exp(-i theta)/sqrt(N) = C + i*S with S = -sin
    C = (np.cos(theta) * scale).astype(np.float32)          # [256,256] symmetric
    S = (-np.sin(theta) * scale).astype(np.float32)         # [256,256] symmetric
    Cw = C[:, :WC].copy()                                   # [256,129]
    Sw = S[:, :WC].copy()                                   # [256,129]
    # inverse real transform along W: out = Xr @ Ar + Xi @ Ai, [129,256]
    kk = np.arange(WC)
    ww = np.arange(W)
    th = 2.0 * np.pi * np.outer(kk, ww) / W
    m = np.full((WC, 1), 2.0, np.float32)
    m[0, 0] = 1.0
    m[WC - 1, 0] = 1.0
    Ar = (m * np.cos(th) * scale).astype(np.float32)        # [129,256]
    Ai = (-m * np.sin(th) * scale).astype(np.float32)       # [129,256]
    Ai[0, :] = 0.0
    Ai[WC - 1, :] = 0.0
    return C, S, Cw, Sw, Ar, Ai


_C, _S, _Cw, _Sw, _Ar, _Ai = _dft_mats()


def _shard_fn_np(mod):
    """Build the per-shard AFNO function with the given numpy-like module
    (jax.numpy on device, or numpy for the CPU fallback)."""
    jnp = mod

    def relu(v):
        return jnp.maximum(v, 0.0)

    def softshrink(v):
        return jnp.sign(v) * jnp.maximum(jnp.abs(v) - LAMBDA, 0.0)

    def fn(x, w1, b1, w2, b2):
        # x: [H, W, BS]; w1: [2, BS, BS]; b1: [2, BS]; w2: [2, BS, BS]; b2: [2, BS]
        bias = x
        # --- rfft over W (axis 1): contract w with Cw/Sw ---
        # x [h, w, c] -> Xr/Xi [h, wc, c]
        xr = jnp.einsum("hwc,wk->hkc", x, _Cw)
        xi = jnp.einsum("hwc,wk->hkc", x, _Sw)
        # --- full DFT over H (axis 0), complex in/out ---
        zr = jnp.einsum("hk,hwc->kwc", _C, xr) - jnp.einsum("hk,hwc->kwc", _S, xi)
        zi = jnp.einsum("hk,hwc->kwc", _C, xi) + jnp.einsum("hk,hwc->kwc", _S, xr)
        # --- block MLP (single 96-channel block on this core) ---
        o1r = relu(zr @ w1[0] - zi @ w1[1] + b1[0])
        o1i = relu(zi @ w1[0] + zr @ w1[1] + b1[1])
        o2r = o1r @ w2[0] - o1i @ w2[1] + b2[0]
        o2i = o1i @ w2[0] + o1r @ w2[1] + b2[1]
        o2r = softshrink(o2r)
        o2i = softshrink(o2i)
        # --- inverse DFT over H: kernel conj = C - i*S ---
        vr = jnp.einsum("kh,kwc->hwc", _C, o2r) + jnp.einsum("kh,kwc->hwc", _S, o2i)
        vi = jnp.einsum("kh,kwc->hwc", _C, o2i) - jnp.einsum("kh,kwc->hwc", _S, o2r)
        # --- inverse rfft over W: out = Vr @ Ar + Vi @ Ai ---
        out = jnp.einsum("hkc,kw->hwc", vr, _Ar) + jnp.einsum("hkc,kw->hwc", vi, _Ai)
        return out + bias

    return fn


def _run_cpu(x, w1, b1, w2, b2):
    fn = _shard_fn_np(np)
    outs = []
    for b in range(NB):
        sl = slice(b * BS, (b + 1) * BS)
        outs.append(fn(x[0, :, :, sl], w1[:, b], b1[:, b], w2[:, b], b2[:, b]))
    return np.concatenate(outs, axis=-1)[None].astype(np.float32)


def _run_neuron(x, w1, b1, w2, b2):
    import jax
    import jax.numpy as jnp

    devs = jax.devices()[:N_CORES]
    if len(devs) < N_CORES:
        raise RuntimeError("need 8 devices")
    fn = _shard_fn_np(jnp)
    pfn = jax.pmap(fn, devices=devs)
    # shard inputs: axis 0 = block/core
    xs = np.moveaxis(x[0].reshape(H, W, NB, BS), 2, 0)       # [8, H, W, BS]
    w1s = np.moveaxis(w1, 1, 0)                               # [8, 2, BS, BS]
    b1s = np.moveaxis(b1, 1, 0)                               # [8, 2, BS]
    w2s = np.moveaxis(w2, 1, 0)
    b2s = np.moveaxis(b2, 1, 0)
    out = pfn(xs, w1s, b1s, w2s, b2s)                         # [8, H, W, BS]
    out = np.asarray(out)
    out = np.moveaxis(out, 0, 2).reshape(1, H, W, HIDDEN)
    return out.astype(np.float32)


def kernel(x, w1, b1, w2, b2):
    x = np.asarray(x, np.float32)
    w1 = np.asarray(w1, np.float32)
    b1 = np.asarray(b1, np.float32)
    w2 = np.asarray(w2, np.float32)
    b2 = np.asarray(b2, np.float32)
    try:
        return _run_neuron(x, w1, b1, w2, b2)
    except Exception:
        return _run_cpu(x, w1, b1, w2, b2)
